# revision 19
# baseline (speedup 1.0000x reference)
"""Distributed Trainium2 Bass kernel for the ACMProxy loss.

Sharding: proxy bank (N=65536) split across 8 NeuronCores, camera-grouped and
evenly dealt so all cores share one SPMD graph. Device does the heavy
(B=64)x(N/8) matmul (bf16 x, fp8 proxies, f32 PSUM) + a max8-based extraction
epilogue; host merges tiny per-core candidate lists exactly (f64 logsumexp,
top-50 of per-chunk top-8s) and computes the B=64 batch terms (MMD/triplet)
in numpy.

v2 (DMA-roofline shaped): proxy slice resident in SBUF, streamed in
consumption order as k-half chunks; h0/h1 matmuls interleaved per k into one
PSUM bank (col-tile positions (0,0)/(0,64)) so the PE chases the DMA front
with slack; epilogue is DVE-only reading PSUM directly (no d_full, no ACT
copies); camera top-8s are per-jtile pieces; one fused output tile flushed in
two DMAs (bulk early, tiny tail).

Device output per core (f32, packed rows p = 64*half + batch_row), layout per
jtile j: [top8 of d+pen_pos | top8 of d+pen_neg | cam pieces (top8 of raw d
per (cam x jtile) intersection, or raw copy when <8 wide)].
"""

import ml_dtypes
import numpy as np

import concourse.mybir as mybir
from concourse import bacc
from concourse.tile import TileContext
from concourse.bass_utils import run_bass_kernel_spmd

# problem constants (hardcoded)
B, D, N = 64, 2048, 65536
M = 8
TEMP = 0.07
NUM_HARDS = 50
LAM_DIS = 0.05
LAM_INS = 0.05
GAMMA = 0.9
NK = 5
MAX_CAMS = 8
NCAMS = 6

PEN = 224.0     # exclusion penalty (exact in bf16/fp8e4m3); real d in [-8, 8]
JT = 512

_cache = {}


# ---------------------------------------------------------------- layout plan
def _jtiles(W):
    """Partition [0, W) into jtiles: 512s up front, two narrow tail tiles so
    the exposed last epilogues (after the final DMA byte) are cheap."""
    tiles = []
    o = 0
    while W - o > 768:
        tiles.append((o, JT))
        o += JT
    rem = W - o  # 8 < rem <= 768 (W is a multiple of 8)
    if rem > 256:
        w1 = ((rem * 3 // 4) + 7) // 8 * 8
        tiles.append((o, w1))
        tiles.append((o + w1, rem - w1))
    else:
        tiles.append((o, rem))
    return tiles


def _plan(cids):
    idx_by_cam = [np.nonzero(cids == c)[0] for c in range(NCAMS)]
    percore = [[idx_by_cam[c][m::M] for c in range(NCAMS)] for m in range(M)]
    slot = [max(len(percore[m][c]) for m in range(M)) for c in range(NCAMS)]
    wA, wB = sum(slot[0:3]), sum(slot[3:6])
    W = ((max(wA, wB) + 7) // 8) * 8
    offs = {}
    o = 0
    for c in range(3):
        offs[c] = o
        o += slot[c]
    o = 0
    for c in range(3, 6):
        offs[c] = o
        o += slot[c]
    njt = len(_jtiles(W))
    return percore, slot, offs, W, njt


def _layout(slot, offs, W, njt):
    """Per-jtile output layout: (pos_off, neg_off, pieces) and total columns.

    pieces: (c, h, lo, hi, off, wout) with lo/hi half-local column coords;
    wout = 8 (max8) or hi-lo when the intersection is narrower than 8.
    """
    jts = _jtiles(W)
    lay = []
    col = 0
    for j, (jo, w) in enumerate(jts):
        if j >= njt - 2:
            # tail tiles ship raw d columns; the host derives candidates
            lay.append(("raw", col, w))
            col += w
            continue
        pos_off = col
        col += 8
        neg_off = col
        col += 8
        pieces = []
        for h in range(2):
            for c in (range(3) if h == 0 else range(3, 6)):
                lo = max(offs[c], jo)
                hi = min(offs[c] + slot[c], jo + w)
                if lo < hi:
                    wout = 8 if hi - lo >= 8 else hi - lo
                    pieces.append((c, h, lo, hi, col, wout))
                    col += wout
        lay.append(("cand", pos_off, neg_off, pieces))
    return lay, col


def _prep_core(m, percore, slot, offs, W, proxy, targets, cams, pids):
    col_g = np.full(2 * W, -1, dtype=np.int64)
    cid_col = np.full(2 * W, -1, dtype=np.int64)
    for c in range(NCAMS):
        h = 0 if c < 3 else 1
        base = h * W + offs[c]
        g = percore[m][c]
        col_g[base:base + len(g)] = g
        cid_col[base:base + slot[c]] = c

    real = col_g >= 0
    proxT = np.zeros((D, 2 * W), dtype=np.float32)
    proxT[:, real] = proxy[col_g[real], :].T
    KT = D // 128
    njt = len(_jtiles(W))

    pid_col = np.where(real, pids[np.where(real, col_g, 0)], -1)
    jts = _jtiles(W)
    ppos = np.zeros((128, W), dtype=ml_dtypes.bfloat16)
    pneg = np.zeros((128, W), dtype=ml_dtypes.bfloat16)
    nposh = np.zeros((128, njt), dtype=np.int64)
    for h in range(2):
        cols = slice(h * W, (h + 1) * W)
        pm = (targets[:, None] == pid_col[None, cols]) & (cams[:, None] != cid_col[None, cols]) & real[None, cols]
        nm = (targets[:, None] != pid_col[None, cols]) & real[None, cols]
        ppos[64 * h:64 * h + 64] = np.where(pm, 0.0, -PEN)
        pneg[64 * h:64 * h + 64] = np.where(nm, 0.0, -PEN)
        for j, (jo, w) in enumerate(jts):
            nposh[64 * h:64 * h + 64, j] = pm[:, jo:jo + w].sum(1)

    # repack jtile-contiguous fp8e4m3: per jtile j, half h: k-major rhs block
    A = proxT.astype(ml_dtypes.float8_e4m3).reshape(KT, 128, 2, W)
    parts = []
    for (jo, w) in jts:
        blk = A[:, :, :, jo:jo + w]                  # (KT,128,2,w)
        for h in range(2):
            parts.append(np.transpose(blk[:, :, h, :], (1, 0, 2)).reshape(128, KT * w))
    proxP = np.ascontiguousarray(np.concatenate(parts, axis=1))
    # masks shipped fused with x (bf16): kernel() appends them
    return {"proxP": proxP, "ppos": np.asarray(ppos), "pneg": np.asarray(pneg)}, nposh


# ---------------------------------------------------------------- bass kernel
def _build(W, njt, slot, offs):
    KT = D // 128
    f32 = mybir.dt.float32
    bf16 = mybir.dt.bfloat16
    nc = bacc.Bacc("TRN2", target_bir_lowering=False, debug=False, num_devices=M)

    lay, ncols = _layout(slot, offs, W, njt)

    fp8 = mybir.dt.float8e4
    proxP_e = nc.dram_tensor("proxP", [128, 2 * KT * W], fp8, kind="ExternalInput").ap()
    xm_e = nc.dram_tensor("xm", [128, KT * B], bf16, kind="ExternalInput").ap()
    pm_e = nc.dram_tensor("pm", [128, 2 * W], fp8, kind="ExternalInput").ap()

    o_all = nc.dram_tensor("o_all", [128, ncols], f32, kind="ExternalOutput").ap()

    with TileContext(nc) as tc:
        with (
            tc.tile_pool(name="const", bufs=1) as constp,
            tc.tile_pool(name="ps", bufs=4, space="PSUM") as psump,
            tc.tile_pool(name="scr", bufs=4) as scrp,
        ):
            xts = constp.tile([128, KT * B], bf16)
            nc.sync.dma_start(out=xts[:], in_=xm_e[:, :])

            prox = constp.tile([128, 2 * KT * W], fp8)
            pm = constp.tile([128, 2 * W], fp8)
            ppos_s = pm[:, 0:W]
            pneg_s = pm[:, W:2 * W]
            outt = constp.tile([128, ncols], f32)

            # --- input DMA stream, in consumption order -------------------
            # k-split chunks alternate h0/h1 so the interleaved pair-matmuls
            # chase the DMA front; fine splits at head (early start) and tail
            # (minimal exposed matmul after the last byte lands).
            jts = _jtiles(W)

            poff = 0
            for j, (jo, w) in enumerate(jts):
                span = KT * w
                if w >= 256 and (j == 0 or j >= njt - 3):
                    # fine k-quarter chunks: early matmul start (j0) and a
                    # minimal exposed matmul tail (last tiles)
                    q = KT // 4
                    for (klo, khi) in [(i * q, (i + 1) * q) for i in range(4)]:
                        for h in range(2):
                            lo = poff + h * span + klo * w
                            hi = poff + h * span + khi * w
                            nc.sync.dma_start(out=prox[:, lo:hi],
                                              in_=proxP_e[:, lo:hi])
                else:
                    # one merged transfer per jtile (both halves, 16KB/part
                    # descriptors) — fewest issues, best HBM efficiency
                    nc.sync.dma_start(out=prox[:, poff:poff + 2 * span],
                                      in_=proxP_e[:, poff:poff + 2 * span])
                if j == 0:  # masks land behind j0's data, before j0's epilogue
                    nc.sync.dma_start(out=pm[:], in_=pm_e[:, :])
                poff += 2 * span

            # --- matmul + DVE epilogue per jtile --------------------------
            poff = 0
            for j, (jo, w) in enumerate(jts):
                jsl = slice(jo, jo + w)
                span = KT * w
                ps = psump.tile([128, JT], f32, tag="ps")
                for k in range(KT):
                    xk = xts[:, k * B:(k + 1) * B]
                    r0 = prox[:, poff + k * w: poff + (k + 1) * w]
                    r1 = prox[:, poff + span + k * w: poff + span + (k + 1) * w]
                    nc.tensor.matmul(ps[0:64, :w], xk, r0,
                                     start=(k == 0), stop=(k == KT - 1))
                    nc.tensor.matmul(ps[64:128, :w], xk, r1,
                                     start=(k == 0), stop=(k == KT - 1))
                poff += 2 * span

                if lay[j][0] == "raw":
                    # tail tiles: one ACT copy of raw d; host derives the
                    # pos/neg/cam candidates for these columns in numpy.
                    _, roff, _ = lay[j]
                    nc.scalar.copy(outt[:, roff:roff + w], ps[:, :w])
                    nc.sync.dma_start(out=o_all[:, roff:roff + w],
                                      in_=outt[:, roff:roff + w])
                    continue

                _, pos_off, neg_off, pieces = lay[j]
                # pos candidates: top-8 of d + pen_pos per (row, half, jtile)
                tp = scrp.tile([128, JT], f32, tag="tp")
                nc.vector.tensor_add(tp[:, :w], ps[:, :w], ppos_s[:, jsl])
                nc.vector.max(outt[:, pos_off:pos_off + 8], tp[:, :w])
                # neg candidates: top-8 of d + pen_neg
                mn = scrp.tile([128, JT], f32, tag="mn")
                nc.vector.tensor_add(mn[:, :w], ps[:, :w], pneg_s[:, jsl])
                nc.vector.max(outt[:, neg_off:neg_off + 8], mn[:, :w])
                # camera pieces: raw d top-8 (or raw copy when <8 wide)
                for (c, h, lo, hi, off, wout) in pieces:
                    pr = slice(64 * h, 64 * h + 64)
                    src = ps[pr, lo - jo:hi - jo]
                    if wout == 8:
                        nc.vector.max(outt[pr, off:off + 8], src)
                    else:
                        nc.vector.tensor_copy(out=outt[pr, off:off + wout], in_=src)

                if j == njt - 3:
                    bulk = lay[njt - 2][1]
                    nc.sync.dma_start(out=o_all[:, 0:bulk], in_=outt[:, 0:bulk])

    nc.compile()
    return nc


# ---------------------------------------------------------------- host math
def _host_batch_terms(x, targets, cams, cids_hist, vals, D_cam):
    """Mirror of reference _acm_dis / _acm_ins with device-supplied `vals`."""
    Bsz = x.shape[0]
    C = MAX_CAMS
    f32 = np.float32

    diff = x[:, None, :] - x[None, :, :]
    d2 = np.sum(diff * diff, axis=-1, dtype=f32)
    eye = np.eye(Bsz, dtype=bool)
    pw = np.sqrt(np.where(eye, f32(1.0), d2)).astype(f32) * (~eye)

    # ---- _acm_dis (MMD between intra/inter camera pair distances)
    iu, ju = np.triu_indices(Bsz, 1)
    dvec = pw[iu, ju].astype(f32)
    same = cams[iu] == cams[ju]
    wx = same.astype(f32)
    wy = (~same).astype(f32)
    n = wx.sum(dtype=f32)
    mm = wy.sum(dtype=f32)
    sq = (dvec[:, None] - dvec[None, :]) ** 2
    Sxx = wx @ sq @ wx
    Syy = wy @ sq @ wy
    denom = max(f32(1.0), n * n - n + mm * mm - mm)
    sigma = max(max(Sxx + Syy, f32(1e-6)) / denom, f32(1e-6))
    K = np.exp(-sq / sigma, dtype=f32)
    kxx = (wx @ K @ wx) / max(n * n, f32(1.0))
    kyy = (wy @ K @ wy) / max(mm * mm, f32(1.0))
    kxy = (wx @ K @ wy) / max(n * mm, f32(1.0))
    dis = (kxx + kyy - 2.0 * kxy) if (n >= 2 and mm >= 2) else f32(0.0)

    # ---- _acm_ins
    Moh = np.zeros((Bsz, C), dtype=f32)
    Moh[np.arange(Bsz), cams] = 1.0
    triu = np.triu(np.ones((Bsz, Bsz), dtype=f32), 1)
    pwt = pw * triu
    intra_sum = np.einsum('ic,jc,ij->c', Moh, Moh, pwt).astype(f32)
    intra_cnt = np.einsum('ic,jc,ij->c', Moh, Moh, triu).astype(f32)
    intra_mean = intra_sum / np.maximum(intra_cnt, 1.0)
    cam_cnt = Moh.sum(0)
    proxy_cnt = cids_hist.astype(f32)
    with np.errstate(invalid='ignore'):
        mean_d = (Moh.T @ vals) / np.maximum(cam_cnt, 1.0)[:, None]

    Dc = D_cam.astype(f32).copy()
    rng = np.arange(C)
    diag = np.diagonal(Dc).copy()
    diag_new = GAMMA * diag + (1.0 - GAMMA) * np.maximum(intra_mean, 1e-6)
    Dc[rng, rng] = np.where(intra_cnt >= 1.0, diag_new, diag)
    present = cam_cnt > 0
    off_mask = present[:, None] & present[None, :] & (proxy_cnt[None, :] > 0) & (~np.eye(C, dtype=bool))
    with np.errstate(invalid='ignore'):
        upd = GAMMA * Dc + (1.0 - GAMMA) * np.maximum(mean_d, 1e-6)
    Dc = np.where(off_mask, upd, Dc)
    Dc = np.maximum(Dc, 1e-6)

    dist_raw = np.maximum(1.0 - x @ x.T, 1e-8).astype(f32)
    pos_m = (targets[:, None] == targets[None, :]) & (cams[:, None] != cams[None, :])
    neg_m = targets[:, None] != targets[None, :]
    hard_pos = np.argmin(np.where(pos_m, dist_raw, np.inf), axis=1)
    hard_neg = np.argmax(np.where(neg_m, dist_raw, -np.inf), axis=1)
    idx = np.arange(Bsz)
    ic = cams
    scale_pos = np.clip(Dc[ic, ic] / Dc[ic, cams[hard_pos]], 0.1, 10.0)
    scale_neg = np.clip(Dc[ic, ic] / Dc[ic, cams[hard_neg]], 0.1, 10.0)
    d_pos = dist_raw[idx, hard_pos] * scale_pos
    d_neg = dist_raw[idx, hard_neg] * scale_neg
    trip = np.maximum(d_pos - d_neg + 0.2, 0.0)
    valid = (pos_m.sum(1) > 0) & (neg_m.sum(1) > 0)
    cnt = f32(valid.sum())
    ins = np.where(valid, trip, 0.0).sum(dtype=f32) / max(cnt, f32(1.0)) if cnt > 0 else f32(0.0)
    return f32(dis), f32(ins)


# ---------------------------------------------------------------- entry point
def kernel(**inputs):
    inp = np.asarray(inputs["inputs"], dtype=np.float32)
    targets = np.asarray(inputs["targets"]).astype(np.int64)
    cams = np.asarray(inputs["cams"]).astype(np.int64)
    proxy = np.asarray(inputs["proxy"], dtype=np.float32)
    pids = np.asarray(inputs["pids"]).astype(np.int64)
    cids = np.asarray(inputs["cids"]).astype(np.int64)
    D_cam = np.asarray(inputs["D_cam"], dtype=np.float32)

    x = inp / np.maximum(np.linalg.norm(inp, axis=1, keepdims=True), 1e-12)
    x = x.astype(np.float32)
    xP = x.T.reshape(D // 128, 128, B).transpose(1, 0, 2).reshape(
        128, (D // 128) * B).astype(ml_dtypes.bfloat16)

    percore, slot, offs, W, njt = _plan(cids)
    lay, ncols = _layout(slot, offs, W, njt)

    key = (W, njt, tuple(slot))
    if key not in _cache:
        _cache[key] = _build(W, njt, slot, offs)
    nc = _cache[key]

    in_maps = []
    nposh_all = []
    masks_all = []
    for m in range(M):
        im, nposh = _prep_core(m, percore, slot, offs, W, proxy, targets, cams, pids)
        ppos = im.pop("ppos")
        pneg = im.pop("pneg")
        im["xm"] = xP
        im["pm"] = np.ascontiguousarray(
            np.concatenate([ppos, pneg], axis=1).astype(ml_dtypes.float8_e4m3))
        in_maps.append(im)
        nposh_all.append(nposh)
        masks_all.append((ppos == 0, pneg == 0))

    res = run_bass_kernel_spmd(nc, in_maps, core_ids=list(range(M)))
    outs = res.results

    # ---------------- merge main loss ----------------
    npos = np.zeros(B, dtype=np.int64)
    pos_cands = [[] for _ in range(B)]
    neg_cands = []
    cam_cands = [[] for _ in range(NCAMS)]
    jts = _jtiles(W)
    for m in range(M):
        nposh = nposh_all[m]
        npos += nposh[0:64].sum(axis=1) + nposh[64:128].sum(axis=1)
        posmask, negmask = masks_all[m]
        oa = np.asarray(outs[m]["o_all"], dtype=np.float32)  # (128, ncols)
        pos_cols = []
        neg_cols = []
        raw_pos = [[] for _ in range(B)]
        for j in range(njt):
            if lay[j][0] == "raw":
                # host-side candidate extraction for raw-dumped tail columns
                _, roff, w = lay[j]
                jo = jts[j][0]
                for h in range(2):
                    rows = slice(64 * h, 64 * h + 64)
                    dh = oa[rows, roff:roff + w]
                    pmj = posmask[rows, jo:jo + w]
                    nmj = negmask[rows, jo:jo + w]
                    for i in range(B):
                        raw_pos[i].append(dh[i][pmj[i]])
                    dn = np.where(nmj, dh, -1e9).astype(np.float32)
                    neg_cols.append(-np.sort(-dn, axis=1)[:, :8])
                    for c in (range(3) if h == 0 else range(3, 6)):
                        lo = max(offs[c], jo)
                        hi = min(offs[c] + slot[c], jo + w)
                        if lo < hi:
                            cam_cands[c].append(dh[:, lo - jo:hi - jo])
                continue
            _, pos_off, neg_off, pieces = lay[j]
            pos_cols.append(oa[0:64, pos_off:pos_off + 8])
            pos_cols.append(oa[64:128, pos_off:pos_off + 8])
            neg_cols.append(oa[0:64, neg_off:neg_off + 8])
            neg_cols.append(oa[64:128, neg_off:neg_off + 8])
            for (c, h, lo, hi, off, wout) in pieces:
                cam_cands[c].append(oa[64 * h:64 * h + 64, off:off + wout])
        pos_pack = np.concatenate(pos_cols, axis=1)
        for i in range(B):
            v = pos_pack[i]
            vals = [v[v > -50.0]] + raw_pos[i]  # pen'd values are <= -200
            pos_cands[i].append(np.concatenate(vals))
        neg_cands.append(np.concatenate(neg_cols, axis=1))

    lse_pos = np.full(B, -np.inf)
    mean_pos = np.zeros(B)
    for i in range(B):
        v = np.concatenate(pos_cands[i]).astype(np.float64)
        assert len(v) == npos[i], (i, len(v), npos[i])
        if len(v):
            t = v / TEMP
            tm = t.max()
            lse_pos[i] = tm + np.log(np.exp(t - tm).sum())
            mean_pos[i] = t.mean()

    allneg = np.concatenate(neg_cands, axis=1)
    top50 = -np.partition(-allneg, NUM_HARDS - 1, axis=1)[:, :NUM_HARDS]
    # guard: a chunk contributing all its kept values to the top-50 means the
    # true top-50 might extend beyond what the device kept.
    kept = 8
    nchunks = allneg.shape[1] // kept
    thr = top50[:, -1][:, None]
    per_chunk = (allneg.reshape(B, nchunks, kept) >= thr[:, :, None]).sum(axis=2)
    assert per_chunk.max() < kept, "neg top-k chunk saturated"
    t = top50 / TEMP
    tm = t.max(axis=1)
    lse_neg = tm + np.log(np.sum(np.exp(t - tm[:, None]), axis=1))

    lse = np.logaddexp(lse_pos, lse_neg)
    row = np.where(npos > 0, lse - mean_pos, 0.0)
    loss = row.sum() / B

    # ---------------- batch terms ----------------
    present = np.zeros(MAX_CAMS)
    np.add.at(present, cams, 1.0)
    multi_cam = (present > 0).sum() >= 2

    if multi_cam:
        vals = np.full((B, MAX_CAMS), np.inf, dtype=np.float32)
        for c in range(NCAMS):
            cand = np.concatenate(cam_cands[c], axis=1)  # (64, *) d units
            top5 = -np.partition(-cand, NK - 1, axis=1)[:, :NK]
            vals[:, c] = np.maximum(1.0 - top5, 1e-8).mean(axis=1)
        cids_hist = np.zeros(MAX_CAMS)
        np.add.at(cids_hist, cids, 1.0)
        dis, ins = _host_batch_terms(x, targets, cams, cids_hist, vals, D_cam)
        loss = loss + LAM_DIS * float(dis) + LAM_INS * float(ins)

    return np.float32(loss)
